# revision 17
# baseline (speedup 1.0000x reference)
"""Trainium2 Bass kernel for nn_BayesianPredictor.

Data-parallel over the batch dim of x across 8 NeuronCores. The tiny
Bayesian-layer params and eps samples are replicated; each core computes
its batch shard of pred_mean / aleatoric / epistemic, plus a partial sum
of the prior/posterior log-prob terms over a host-assigned slice of the
weight rows. Host combines the scalar partials (the only "all-reduce").

Per-core math (2048-row batch shard):
  sigma = log1p(exp(rho));  w_n = mu + sigma * eps_n
  L_n   = x @ w_n.T  computed as [c, b] psum tiles (fp32r matmuls,
          moving dim 512), contraction over k-tiles of transposed x/w
  e_n   = exp(L_n + b_n)   (ACT, bias per-partition; no max-subtraction:
          |logits| < ~30 so exp stays in fp32 range)
  p_n   = e_n / sum_c e_n  after PE-transposing e to [b, c] tiles
  pred  = x @ wbar.T + bbar   (mean over passes is linear)
  alea  = mean_n (sum_c p_n - sum_c p_n^2)
  epis  = mean_n sum_c p_n^2 - sum_c pbar^2
sum_c p_n comes fused from the normalize tensor_scalar; sum_c p_n^2 from
bn_stats moments; pbar accumulates via gpsimd accumulate-DMAs.
"""

import math
import numpy as np

import concourse.bass as bass
import concourse.bacc as bacc
import concourse.mybir as mybir
from concourse.tile import TileContext

# Problem constants
B = 16384
IN_DIM = 2048
C = 101
NPASS = 10
NCORES = 8
PI = 0.5
LOG_SQRT_2PI = 0.5 * math.log(2.0 * math.pi)
SIGMA2 = math.exp(-6.0)
K2 = 1.0 / (2.0 * SIGMA2 * SIGMA2)            # 0.5 * e^12
C1 = math.log(PI) - LOG_SQRT_2PI              # folds pi into N(0, 1) exp
C2 = math.log(1.0 - PI) - LOG_SQRT_2PI + 6.0  # folds (1-pi) into N(0, s2) exp

ROWS_SH = 13          # ceil(101/8) prior-shard rows per core (zero padded)

F32 = mybir.dt.float32
F32R = mybir.dt.float32r
BF16 = mybir.dt.bfloat16


def build_program(BC=B // NCORES, I=IN_DIM, NP=NPASS, mode="f32r"):
    """Emit the single-core Bass program (run SPMD across the 8 cores)."""
    KB = I // 128           # contraction k-tiles
    TB = BC // 128          # batch 128-row tiles per core
    CH = 512 if BC >= 512 else BC
    NCH = BC // CH          # matmul chunks per core
    G = CH // 128           # 128-row groups per chunk
    SH_F = ROWS_SH * I // 128
    NE = float((C + 1) // 2)  # bn_stats even-element count
    NO = float(C // 2)        # bn_stats odd-element count

    mm_store = {"bf16": BF16, "f32r": F32R, "f32": F32}[mode]

    nc = bacc.Bacc()

    # ---------------- I/O ----------------
    x_in = nc.declare_dram_parameter("x", [BC, I], F32, isOutput=False)
    wrho_in = nc.declare_dram_parameter("wrho", [C, I], F32, isOutput=False)
    bmu_in = nc.declare_dram_parameter("bmu", [C, 1], F32, isOutput=False)
    brho_in = nc.declare_dram_parameter("brho", [C, 1], F32, isOutput=False)
    epsbT_in = nc.declare_dram_parameter("epsbT", [C, NP], F32, isOutput=False)
    epsw_in = nc.declare_dram_parameter("epsw", [NP, C, I], F32, isOutput=False)
    ident_in = nc.declare_dram_parameter("ident", [128, 128], F32, isOutput=False)
    ones_in = nc.declare_dram_parameter("ones", [128, 1], F32, isOutput=False)
    musW_in = nc.declare_dram_parameter("mu_sh", [128, SH_F], F32, isOutput=False)
    rhosW_in = nc.declare_dram_parameter("rho_sh", [128, SH_F], F32, isOutput=False)
    mask_in = nc.declare_dram_parameter("mask_sh", [128, SH_F], F32, isOutput=False)
    epssh_in = nc.declare_dram_parameter("eps_sh", [NP, 128, SH_F], F32,
                                         isOutput=False)

    pred_out = nc.declare_dram_parameter("pred", [BC, C], F32, isOutput=True)
    alea_out = nc.declare_dram_parameter("alea", [BC], F32, isOutput=True)
    epis_out = nc.declare_dram_parameter("epis", [BC], F32, isOutput=True)
    scal_out = nc.declare_dram_parameter("scal", [8], F32, isOutput=True)

    AF = mybir.ActivationFunctionType
    ALU = mybir.AluOpType
    AX = mybir.AxisListType

    with TileContext(nc) as tc:
        with (
            tc.tile_pool(name="big", bufs=1) as big,      # persistent singletons
            tc.tile_pool(name="nat", bufs=2) as natp,     # [C, I] naturals
            tc.tile_pool(name="wt", bufs=2) as wtp,       # [128, KB*C] weightT
            tc.tile_pool(name="q512", bufs=4) as q512,    # [<=128, 512] rotating
            tc.tile_pool(name="psb", bufs=3) as psbp,     # [128, TB*C] p-pass
            tc.tile_pool(name="stat", bufs=2) as statp,   # per-chunk stats
            tc.tile_pool(name="smal", bufs=1) as smal,    # tiny singletons
            tc.tile_pool(name="prk", bufs=1) as prk,      # prior persistents
            tc.tile_pool(name="prs", bufs=3) as prs,      # prior scratch
            tc.tile_pool(name="pre", bufs=2) as prep,     # prior eps staging
            tc.tile_pool(name="ps_tr", bufs=2, space="PSUM") as ps_tr,
            tc.tile_pool(name="ps_L", bufs=2, space="PSUM") as ps_L,
            tc.tile_pool(name="ps_e", bufs=2, space="PSUM") as ps_e,
            tc.tile_pool(name="ps_m", bufs=1, space="PSUM") as ps_m,
        ):
            # ---------------- Phase 0: params ----------------
            ident = big.tile([128, 128], F32, tag="ident")
            nc.sync.dma_start(ident[:], ident_in[:])
            ones = big.tile([128, 1], F32, tag="ones")
            nc.sync.dma_start(ones[:], ones_in[:])
            cb1 = big.tile([128, 1], F32, tag="cb1")
            nc.vector.memset(cb1[:], C1)
            cb2 = big.tile([128, 1], F32, tag="cb2")
            nc.vector.memset(cb2[:], C2)

            sig = natp.tile([C, I], F32, tag="nat")   # starts as rho
            nc.sync.dma_start(sig[:], wrho_in[:])
            bmu = smal.tile([C, 1], F32, tag="bmu")
            nc.sync.dma_start(bmu[:], bmu_in[:])
            sigb = smal.tile([C, 1], F32, tag="sigb")   # starts as brho
            nc.sync.dma_start(sigb[:], brho_in[:])
            m_sh = prk.tile([128, SH_F], F32, tag="m_sh")
            nc.sync.dma_start(m_sh[:], musW_in[:])
            mask_sh = prk.tile([128, SH_F], F32, tag="mask_sh")
            nc.sync.dma_start(mask_sh[:], mask_in[:])
            sig_sh = prk.tile([128, SH_F], F32, tag="sig_sh")
            nc.sync.dma_start(sig_sh[:], rhosW_in[:])

            # sigma = log1p(exp(rho)) everywhere; exps grouped, then lns
            nc.scalar.activation(sig[:], sig[:], AF.Exp)
            nc.scalar.activation(sigb[:], sigb[:], AF.Exp)
            nc.scalar.activation(sig_sh[:], sig_sh[:], AF.Exp)
            nc.vector.tensor_scalar_add(sig[:], sig[:], 1.0)
            nc.vector.tensor_scalar_add(sigb[:], sigb[:], 1.0)
            nc.vector.tensor_scalar_add(sig_sh[:], sig_sh[:], 1.0)
            nc.scalar.activation(sig[:], sig[:], AF.Ln)
            nc.scalar.activation(sigb[:], sigb[:], AF.Ln)
            nc.scalar.activation(sig_sh[:], sig_sh[:], AF.Ln)
            logsig = prs.tile([128, SH_F], F32, tag="pr_scr")
            nc.scalar.activation(logsig[:], sig_sh[:], AF.Ln)
            logsb = smal.tile([C, 1], F32, tag="logsb")
            nc.scalar.activation(logsb[:], sigb[:], AF.Ln)

            # masked sum of log sigma over the shard
            scrp = prs.tile([128, SH_F], F32, tag="pr_scr")
            nc.vector.tensor_mul(scrp[:], logsig[:], mask_sh[:])
            ls_col = smal.tile([128, 1], F32, tag="ls_col")
            nc.vector.tensor_reduce(ls_col[:], scrp[:], axis=AX.X, op=ALU.add)

            epsbT = smal.tile([C, NP], F32, tag="epsbT")
            nc.sync.dma_start(epsbT[:], epsbT_in[:])
            b_cols = smal.tile([C, NP], F32, tag="b_cols")
            nc.vector.tensor_scalar(
                out=b_cols[:], in0=epsbT[:], scalar1=sigb[:], scalar2=bmu[:],
                op0=ALU.mult, op1=ALU.add)

            # transposed sigma: [128, KB*C], k-tile k at cols [k*C,(k+1)*C)
            sigT = big.tile([128, KB * C], mm_store, tag="sigT")
            for kg in range(0, KB, 4):
                ng = min(4, KB - kg)
                ps = ps_tr.tile([128, 512], F32, tag="ps_tr")
                for t in range(ng):
                    nc.tensor.transpose(
                        ps[:, t * C:(t + 1) * C],
                        sig[:, (kg + t) * 128:(kg + t + 1) * 128],
                        ident[0:C, 0:C])
                nc.scalar.copy(sigT[:, kg * C:(kg + ng) * C], ps[:, 0:ng * C])

            # ---------------- Phase 1: transpose x into xT ----------------
            # xT: [128, KB*BC]; k-tile k at cols [k*BC,(k+1)*BC), col index = b
            xT = big.tile([128, KB * BC], mm_store, tag="xT")
            PAIR = 2
            mporder = sorted(range(TB // PAIR), key=lambda m: m * PAIR * 128 // CH)
            for m in mporder:
                for q in range(KB // 4):
                    xs = []
                    for r in range(PAIR):
                        t = q512.tile([128, 512], F32, tag="q512")
                        nc.sync.dma_start(
                            t[:], x_in[(m * PAIR + r) * 128:(m * PAIR + r + 1) * 128,
                                       q * 512:(q + 1) * 512])
                        xs.append(t)
                    for kk in range(4):
                        k = q * 4 + kk
                        ps = ps_tr.tile([128, 512], F32, tag="ps_tr")
                        for r in range(PAIR):
                            nc.tensor.transpose(
                                ps[:, r * 128:(r + 1) * 128],
                                xs[r][:, kk * 128:(kk + 1) * 128],
                                ident[:])
                        dst = xT[:, k * BC + m * PAIR * 128:
                                 k * BC + (m + 1) * PAIR * 128]
                        if k % 4 == 3:
                            nc.vector.tensor_copy(dst, ps[:, 0:PAIR * 128])
                        else:
                            nc.scalar.copy(dst, ps[:, 0:PAIR * 128])

            # ebar accumulation (background, consumed by pred path at the end)
            ebar = natp.tile([C, I], F32, tag="nat")
            for n in range(NP):
                nc.gpsimd.dma_start(
                    out=ebar[:], in_=epsw_in[n],
                    accum_op=(ALU.bypass if n == 0 else ALU.add))

            PRI_S = prk.tile([128, NP * SH_F], F32, tag="PRI_S")
            EW2 = smal.tile([128, NP], F32, tag="EW2")
            # stats in n-major layout: column n*TB + t
            SP = big.tile([128, NP * TB], F32, tag="SP")
            SP2 = big.tile([128, NP * TB], F32, tag="SP2")
            pbar = big.tile([128, TB * C], F32, tag="pbar")

            # ---------------- Phase 3: pass loop ----------------
            for n in range(NP):
                # w_nT = sigT (.) epsT (mu pre-folded into eps on host)
                wT = wtp.tile([128, KB * C], mm_store, tag="wt")
                for q in range(KB // 4):
                    eq = q512.tile([C, 512], F32, tag="q512")
                    nc.sync.dma_start(eq[:], epsw_in[n][:, q * 512:(q + 1) * 512])
                    ps = ps_tr.tile([128, 512], F32, tag="ps_tr")
                    for kk in range(4):
                        nc.tensor.transpose(
                            ps[:, kk * C:(kk + 1) * C],
                            eq[:, kk * 128:(kk + 1) * 128],
                            ident[0:C, 0:C])
                    nc.vector.tensor_mul(
                        wT[:, q * 4 * C:(q + 1) * 4 * C], ps[:, 0:4 * C],
                        sigT[:, q * 4 * C:(q + 1) * 4 * C])

                CPH = max(1, NCH // 2)  # chunks per staging half
                HB = CPH * G * C
                p_half = None
                for j in range(NCH):
                    if j % CPH == 0:
                        p_half = psbp.tile([128, HB], F32, tag="psb")
                    psl = ps_L.tile([C, CH], F32, tag="ps_L")
                    for k in range(KB):
                        nc.tensor.matmul(
                            psl[:], wT[:, k * C:(k + 1) * C],
                            xT[:, k * BC + j * CH:k * BC + (j + 1) * CH],
                            start=(k == 0), stop=(k == KB - 1))
                    e_sb = q512.tile([C, CH], F32, tag="q512")
                    nc.scalar.activation(
                        e_sb[:], psl[:], AF.Exp, bias=b_cols[:, n:n + 1])
                    pse = ps_e.tile([128, G * C], F32, tag="ps_e")
                    for g in range(G):
                        nc.tensor.transpose(
                            pse[:, g * C:(g + 1) * C],
                            e_sb[:, g * 128:(g + 1) * 128],
                            ident[0:C, 0:C])
                    s4 = statp.tile([128, G], F32, tag="s4")
                    nc.vector.tensor_reduce(
                        s4[:], pse.rearrange("p (g c) -> p g c", c=C),
                        axis=AX.X, op=ALU.add)
                    r4 = statp.tile([128, G], F32, tag="r4")
                    nc.vector.reciprocal(r4[:], s4[:])
                    pj = p_half[:, (j % CPH) * G * C:(j % CPH + 1) * G * C]
                    cc = n * TB + j * G
                    # p = e * r (broadcast r along c), sum_c p = s * r
                    nc.vector.tensor_tensor(
                        out=pj.rearrange("p (g c) -> p g c", c=C),
                        in0=pse.rearrange("p (g c) -> p g c", c=C),
                        in1=r4.rearrange("p (g o) -> p g o", o=1)
                            .broadcast_to([128, G, C]),
                        op=ALU.mult)
                    nc.vector.tensor_mul(SP[:, cc:cc + G], s4[:], r4[:])
                    # sum_c p^2: square (gpsimd) then segmented reduce
                    p2 = q512.tile([128, G * C], F32, tag="q512")
                    nc.gpsimd.tensor_mul(p2[:], pj[:], pj[:])
                    nc.vector.tensor_reduce(
                        SP2[:, cc:cc + G], p2.rearrange("p (g c) -> p g c", c=C),
                        axis=AX.X, op=ALU.add)
                    if (j + 1) % CPH == 0:
                        h = j // CPH
                        nc.gpsimd.dma_start(
                            out=pbar[:, h * HB:(h + 1) * HB], in_=p_half[:],
                            accum_op=(ALU.bypass if n == 0 else ALU.add))

                # prior-shard contribution of pass n (Ln deferred)
                esh = prep.tile([128, SH_F], F32, tag="pr_eps")
                nc.sync.dma_start(esh[:], epssh_in[n])
                wsh = prs.tile([128, SH_F], F32, tag="pr_scr")
                nc.vector.tensor_mul(wsh[:], esh[:], sig_sh[:])
                nc.vector.tensor_mul(wsh[:], wsh[:], wsh[:])   # q = w^2
                e1 = prs.tile([128, SH_F], F32, tag="pr_scr")
                nc.scalar.activation(e1[:], wsh[:], AF.Exp, bias=cb1[:], scale=-0.5)
                e2 = prs.tile([128, SH_F], F32, tag="pr_scr")
                nc.scalar.activation(e2[:], wsh[:], AF.Exp, bias=cb2[:], scale=-K2)
                nc.vector.tensor_add(
                    PRI_S[:, n * SH_F:(n + 1) * SH_F], e1[:], e2[:])
                scrn = prs.tile([128, SH_F], F32, tag="pr_scr")
                nc.vector.tensor_sub(scrn[:], esh[:], m_sh[:])
                nc.vector.tensor_mul(scrn[:], scrn[:], scrn[:])
                nc.vector.tensor_reduce(
                    EW2[:, n:n + 1], scrn[:], axis=AX.X, op=ALU.add)

            # ---------------- Phase 2: pred path ----------------
            nc.vector.tensor_scalar_mul(ebar[:], ebar[:], 1.0 / NP)
            wbarT = wtp.tile([128, KB * C], mm_store, tag="wt")
            for kg in range(0, KB, 4):
                ng = min(4, KB - kg)
                ps = ps_tr.tile([128, 512], F32, tag="ps_tr")
                for t in range(ng):
                    nc.tensor.transpose(
                        ps[:, t * C:(t + 1) * C],
                        ebar[:, (kg + t) * 128:(kg + t + 1) * 128],
                        ident[0:C, 0:C])
                nc.vector.tensor_mul(
                    wbarT[:, kg * C:(kg + ng) * C], ps[:, 0:ng * C],
                    sigT[:, kg * C:(kg + ng) * C])

            bbar = smal.tile([C, 1], F32, tag="bbar")
            nc.vector.tensor_reduce(bbar[:], b_cols[:], axis=AX.X, op=ALU.add)
            nc.vector.tensor_scalar_mul(bbar[:], bbar[:], 1.0 / NP)

            pm_bc = wtp.tile([128, TB * C], F32, tag="wt")
            for j in range(NCH):
                psm = ps_L.tile([C, CH], F32, tag="ps_L")
                for k in range(KB):
                    nc.tensor.matmul(
                        psm[:], wbarT[:, k * C:(k + 1) * C],
                        xT[:, k * BC + j * CH:k * BC + (j + 1) * CH],
                        start=(k == 0), stop=(k == KB - 1))
                pm_cb = q512.tile([C, CH], F32, tag="q512")
                nc.scalar.activation(pm_cb[:], psm[:], AF.Identity, bias=bbar[:])
                pse = ps_e.tile([128, G * C], F32, tag="ps_e")
                for g in range(G):
                    nc.tensor.transpose(
                        pse[:, g * C:(g + 1) * C],
                        pm_cb[:, g * 128:(g + 1) * 128],
                        ident[0:C, 0:C])
                nc.vector.tensor_copy(pm_bc[:, j * G * C:(j + 1) * G * C], pse[:])
            nc.sync.dma_start(
                pred_out.rearrange("(t p) c -> p t c", p=128),
                pm_bc.rearrange("p (t c) -> p t c", c=C))

            # ---------------- Phase 4: scalar outputs ----------------
            # prior: one Ln over all passes, then masked sum
            nc.scalar.activation(PRI_S[:], PRI_S[:], AF.Ln)
            nc.vector.tensor_tensor(
                out=PRI_S.rearrange("p (n f) -> p n f", f=SH_F),
                in0=PRI_S.rearrange("p (n f) -> p n f", f=SH_F),
                in1=mask_sh.rearrange("p (o f) -> p o f", o=1)
                    .broadcast_to([128, NP, SH_F]),
                op=ALU.mult)
            pri_col = smal.tile([128, 1], F32, tag="pri_col")
            nc.vector.tensor_reduce(
                pri_col[:], PRI_S.rearrange("p (n f) -> p n f", f=SH_F),
                axis=AX.XY, op=ALU.add)
            ew2_col = smal.tile([128, 1], F32, tag="ew2_col")
            nc.vector.tensor_reduce(ew2_col[:], EW2[:], axis=AX.X, op=ALU.add)
            stk = smal.tile([128, 3], F32, tag="stk")
            nc.vector.tensor_copy(stk[:, 0:1], pri_col[:])
            nc.vector.tensor_copy(stk[:, 1:2], ls_col[:])
            nc.vector.tensor_copy(stk[:, 2:3], ew2_col[:])
            ps3 = ps_m.tile([3, 1], F32, tag="ps_m")
            nc.tensor.matmul(ps3[:], stk[:], ones[:], start=True, stop=True)
            sc3 = smal.tile([3, 1], F32, tag="sc3")
            nc.vector.tensor_copy(sc3[:], ps3[:])
            nc.sync.dma_start(scal_out[0:3], sc3[:])

            # bias prior/posterior pieces (tiny; host uses core 0's values)
            qb = smal.tile([C, NP], F32, tag="qb")
            nc.vector.tensor_mul(qb[:], b_cols[:], b_cols[:])
            e1b = smal.tile([C, NP], F32, tag="e1b")
            nc.scalar.activation(e1b[:], qb[:], AF.Exp, bias=cb1[0:C, :],
                                 scale=-0.5)
            e2b = smal.tile([C, NP], F32, tag="e2b")
            nc.scalar.activation(e2b[:], qb[:], AF.Exp, bias=cb2[0:C, :],
                                 scale=-K2)
            nc.vector.tensor_add(e1b[:], e1b[:], e2b[:])
            nc.scalar.activation(e1b[:], e1b[:], AF.Ln)
            mb_red = smal.tile([C, 1], F32, tag="mb_red")
            nc.vector.tensor_reduce(mb_red[:], e1b[:], axis=AX.X, op=ALU.add)
            eb2 = smal.tile([C, 1], F32, tag="eb2")
            scrb = smal.tile([C, NP], F32, tag="scrb")
            nc.vector.tensor_mul(scrb[:], epsbT[:], epsbT[:])
            nc.vector.tensor_reduce(eb2[:], scrb[:], axis=AX.X, op=ALU.add)
            stkb = smal.tile([C, 3], F32, tag="stkb")
            nc.vector.tensor_copy(stkb[:, 0:1], mb_red[:])
            nc.vector.tensor_copy(stkb[:, 1:2], logsb[:])
            nc.vector.tensor_copy(stkb[:, 2:3], eb2[:])
            ps3b = ps_m.tile([3, 1], F32, tag="ps_m")
            nc.tensor.matmul(ps3b[:], stkb[:], ones[0:C, :], start=True, stop=True)
            sc3b = smal.tile([3, 1], F32, tag="sc3b")
            nc.vector.tensor_copy(sc3b[:], ps3b[:])
            nc.sync.dma_start(scal_out[3:6], sc3b[:])

            # ---------------- Phase 5: aleatoric / epistemic ----------------
            dall = smal.tile([128, NP * TB], F32, tag="dall")
            nc.vector.tensor_sub(dall[:], SP[:], SP2[:])
            a16 = smal.tile([128, TB], F32, tag="a16")
            nc.vector.tensor_reduce(
                a16[:], dall.rearrange("p (n t) -> p t n", t=TB),
                axis=AX.X, op=ALU.add)
            nc.vector.tensor_scalar_mul(a16[:], a16[:], 1.0 / NP)

            e16 = smal.tile([128, TB], F32, tag="e16")
            nc.vector.tensor_reduce(
                e16[:], SP2.rearrange("p (n t) -> p t n", t=TB),
                axis=AX.X, op=ALU.add)
            nc.vector.tensor_scalar_mul(e16[:], e16[:], 1.0 / NP)
            qs16 = smal.tile([128, TB], F32, tag="qs16")
            for t in range(TB):
                scr = q512.tile([128, C], F32, tag="q512")
                nc.vector.tensor_mul(
                    scr[:], pbar[:, t * C:(t + 1) * C], pbar[:, t * C:(t + 1) * C])
                nc.vector.tensor_reduce(
                    qs16[:, t:t + 1], scr[:], axis=AX.X, op=ALU.add)
            nc.vector.tensor_scalar_mul(qs16[:], qs16[:], 1.0 / (NP * NP))
            nc.vector.tensor_sub(e16[:], e16[:], qs16[:])

            for src, dstd in ((a16, alea_out), (e16, epis_out)):
                pst = ps_tr.tile([128, 512], F32, tag="ps_tr")
                nc.tensor.transpose(pst[0:TB, 0:128], src[:], ident[:])
                ot = smal.tile([TB, 128], F32, tag="ot")
                nc.vector.tensor_copy(ot[:], pst[0:TB, 0:128])
                nc.sync.dma_start(dstd.rearrange("(t p) -> t p", p=128), ot[:])

    nc.compile()
    return nc


def make_core_inputs(x, weight_mu, weight_rho, bias_mu, bias_rho, eps_w, eps_b,
                     BC=B // NCORES, I=IN_DIM, NP=NPASS, ncores=NCORES):
    """Host-side shard: list of per-core input dicts."""
    f32 = np.float32
    x = np.asarray(x, f32)
    weight_mu = np.asarray(weight_mu, f32)
    weight_rho = np.asarray(weight_rho, f32)
    bias_mu = np.asarray(bias_mu, f32).reshape(C, 1)
    bias_rho = np.asarray(bias_rho, f32).reshape(C, 1)
    eps_w = np.asarray(eps_w, f32)
    eps_b = np.asarray(eps_b, f32)

    SH_F = ROWS_SH * I // 128
    ident = np.eye(128, dtype=f32)
    ones = np.ones((128, 1), f32)
    epsbT = np.ascontiguousarray(eps_b.T)  # [C, NP]

    sig_np = np.log1p(np.exp(weight_rho.astype(np.float64))).astype(f32)
    msig = (weight_mu / sig_np).astype(f32)
    eps_p = (eps_w + msig[None]).astype(f32)
    common = dict(wrho=weight_rho, bmu=bias_mu, brho=bias_rho,
                  epsbT=epsbT, epsw=eps_p, ident=ident, ones=ones)

    in_maps = []
    for ci in range(ncores):
        r0 = min(ROWS_SH * ci, C)
        r1 = min(r0 + ROWS_SH, C)
        nr = r1 - r0
        mu_sh = np.zeros((ROWS_SH, I), f32)   # m = mu/sigma
        rho_sh = np.zeros((ROWS_SH, I), f32)
        mask_sh = np.zeros((ROWS_SH, I), f32)
        eps_sh = np.zeros((NP, ROWS_SH, I), f32)
        mu_sh[:nr] = msig[r0:r1]
        rho_sh[:nr] = weight_rho[r0:r1]
        mask_sh[:nr] = 1.0
        eps_sh[:, :nr] = eps_p[:, r0:r1]
        in_maps.append(dict(
            common,
            x=np.ascontiguousarray(x[ci * BC:(ci + 1) * BC]),
            mu_sh=mu_sh.reshape(128, SH_F),
            rho_sh=rho_sh.reshape(128, SH_F),
            mask_sh=mask_sh.reshape(128, SH_F),
            eps_sh=eps_sh.reshape(NP, 128, SH_F),
        ))
    return in_maps


def combine_outputs(results, NP=NPASS, n_w=C * IN_DIM):
    """Host-side unshard + scalar reduce."""
    f32 = np.float32
    pred = np.concatenate([np.asarray(r["pred"]) for r in results], axis=0)
    alea = np.concatenate([np.asarray(r["alea"]) for r in results], axis=0)
    epis = np.concatenate([np.asarray(r["epis"]) for r in results], axis=0)

    sc = np.stack([np.asarray(r["scal"]) for r in results]).astype(np.float64)
    prior_w = sc[:, 0].sum()
    logsig_w = sc[:, 1].sum()
    epsw2 = sc[:, 2].sum()
    prior_b, logsig_b, epsb2 = sc[0, 3], sc[0, 4], sc[0, 5]

    log_prior = (prior_w + prior_b) / NP
    log_post = (-(n_w + C) * LOG_SQRT_2PI
                - (logsig_w + logsig_b)
                - 0.5 * (epsw2 + epsb2) / NP)
    return (pred.astype(f32), np.float32(log_prior), np.float32(log_post),
            alea.astype(f32), epis.astype(f32))


_COMPILED = {}


def kernel(x, weight_mu, weight_rho, bias_mu, bias_rho, eps_w, eps_b,
           npass=NPASS):
    from concourse.bass_utils import run_bass_kernel_spmd

    if "main" not in _COMPILED:
        _COMPILED["main"] = build_program()
    nc = _COMPILED["main"]

    in_maps = make_core_inputs(x, weight_mu, weight_rho, bias_mu, bias_rho,
                               eps_w, eps_b)
    res = run_bass_kernel_spmd(nc, in_maps, list(range(NCORES)))
    return combine_outputs(res.results)
